# revision 66
# baseline (speedup 1.0000x reference)
"""BasicTransformer Trainium2 kernel (Bass/Tile), data-parallel over batch on 8 cores.

Per batch b (one NeuronCore each), all matmul operands fp16 (fp32 PSUM accum):
    M   = W_q^T @ W_k                 (512,512)  once per core, f32r
    Wt  = (lin_w @ W_v)^T             (512,512)  once per core (folds the
                                      post-attention Linear into the V path)
    e   = embed[x]                    (T, D)     indirect-DMA gather
    G   = e @ M                       ([d,t] layout)  so scores = G e^T = q^T k
    Vt  = e @ Wt                      ([t,a] layout)  = v @ lin_w^T
    S   = G^T-slices . E              PE -> PSUM [128, T] per 128-query chunk
    p   = exp(S*scale - rowmax)       DVE rowmax + ACT exp (accum -> l), fp16
    z   = relu((p^T-transposed @ Vt) / l + lin_b)   PE + DVE scale + ACT relu,
                                      accumulated over t per 512-block
    out = sigmoid(clf_w . mean + clf_b)

All tile pools are created ONCE and shared by every timing iteration: separate
per-iteration pools get re-allocated at shifted addresses, and the tile
framework tracks ring reuse only against previous writers (not readers), which
lets the scheduler interleave iteration k+1's early writes over iteration k's
still-live tiles.  With shared pools every cross-iteration overwrite is
value-identical, and explicit Ring WAR guards order slot reuse.

t-order inside the kernel is a fixed permutation of the true t-order; the
computation is permutation-invariant over t (softmax over keys, p@v
contraction, mean over t), so the final (1,) output is unaffected.
"""

import math
import os

import numpy as np

B, T, D, VOCAB = 8, 2048, 512, 32000
P = 128
TC = T // P          # 16 t-chunks
DC = D // P          # 4 d-chunks
NB = T // 512        # 4 blocks of 512 along t
SCALE = 1.0 / math.sqrt(D)
N_CORES = 8

_COMPILED = {}


def _build(iters=1, mm_dtype=None):
    import concourse.bacc as bacc
    import concourse.mybir as mybir
    import concourse.tile as tile
    import concourse.bass as bass
    import bass_rust
    from concourse.masks import make_identity

    dt = mybir.dt
    AF = mybir.ActivationFunctionType
    AX = mybir.AxisListType
    ALU = mybir.AluOpType

    nc = bacc.Bacc("TRN2", target_bir_lowering=False, debug=False)

    x_d = nc.declare_dram_parameter("x", [T], dt.int32, isOutput=False)
    emb_d = nc.declare_dram_parameter("embed", [VOCAB + 1, D], dt.float32, isOutput=False)
    wq_d = nc.declare_dram_parameter("W_q", [D, D], dt.float32r, isOutput=False)
    wk_d = nc.declare_dram_parameter("W_k", [D, D], dt.float32r, isOutput=False)
    wv_d = nc.declare_dram_parameter("W_v", [D, D], dt.float32r, isOutput=False)
    lw_d = nc.declare_dram_parameter("lin_w", [D, D], dt.float32r, isOutput=False)
    lb_d = nc.declare_dram_parameter("lin_b", [D], dt.float32, isOutput=False)
    cw_d = nc.declare_dram_parameter("clf_w", [D], dt.float32, isOutput=False)
    cb_d = nc.declare_dram_parameter("clf_b", [1], dt.float32, isOutput=False)
    out_d = nc.declare_dram_parameter("out", [iters, 1], dt.float32, isOutput=True)
    fence_d = nc.declare_dram_parameter("fence", [iters, 1, 64], dt.float32,
                                        isOutput=True)

    def dep(winst, rinst, why):
        bass_rust.add_dep_helper(winst.ins, rinst.ins, sync=True, reason=why)

    class Ring:
        """Explicit WAR guards for tile-pool ring reuse (the framework chains
        reuse only on previous writers, not their readers)."""

        def __init__(self, bufs):
            self.bufs = bufs
            self.readers = []
            self.n = 0

        def writer(self, winst):
            j = self.n - self.bufs
            self.n += 1
            if 0 <= j < len(self.readers):
                rs = self.readers[j]
                for r in (rs if isinstance(rs, list) else [rs]):
                    if r is not None:
                        dep(winst, r, "ring WAR")
            return winst

        def reader(self, rinst):
            self.readers.append(rinst)
            return rinst

    with tile.TileContext(nc) as tc:
        with tc.tile_pool(name="const", bufs=1) as cpool, \
             tc.tile_pool(name="persist", bufs=1) as pp, \
             tc.tile_pool(name="wsb", bufs=1) as wp, \
             tc.tile_pool(name="etf_pool", bufs=8) as efp, \
             tc.tile_pool(name="e16_pool", bufs=5) as e16p, \
             tc.tile_pool(name="pbuf", bufs=2) as ppb, \
             tc.tile_pool(name="pt_sb", bufs=2) as ptp, \
             tc.tile_pool(name="scratch", bufs=2) as scr, \
             tc.tile_pool(name="s_ps", bufs=1, space="PSUM") as sps, \
             tc.tile_pool(name="t_ps", bufs=2, space="PSUM") as tps, \
             tc.tile_pool(name="z_ps", bufs=2, space="PSUM") as zps:

            ident = cpool.tile([P, P], dt.float32, tag="ident", name="ident")
            make_identity(nc, ident[:])
            identr = cpool.tile([P, P], dt.float32r, tag="identr", name="identr")
            nc.vector.tensor_copy(identr[:], ident[:])
            ident16 = cpool.tile([P, P], dt.float16, tag="ident16", name="ident16")
            nc.vector.tensor_copy(ident16[:], ident[:])

            E16 = pp.tile([P, DC, T], dt.float16, tag="e16", name="e16")
            G16 = pp.tile([P, DC, T], dt.float16, tag="g16", name="g16")
            V16 = pp.tile([P, TC, 512], dt.float16, tag="v16", name="v16")
            M16 = pp.tile([P, DC, D], dt.float16, tag="m16", name="m16")
            Wt16 = pp.tile([P, DC, D], dt.float16, tag="wt16", name="wt16")
            Lbc = pp.tile([P, T], dt.float16, tag="lbc", name="lbc")
            Linv = pp.tile([P, TC], dt.float32, tag="linv", name="linv")
            Linv16 = pp.tile([P, TC], dt.float16, tag="linv16", name="linv16")
            linb = pp.tile([P, DC], dt.float32, tag="linb", name="linb")
            clfw = pp.tile([P, DC], dt.float32, tag="clfw", name="clfw")
            clfb = pp.tile([1, 1], dt.float32, tag="clfb", name="clfb")
            zsum = [pp.tile([P, NB], dt.float32, tag=f"zs{d}", name=f"zs{d}")
                    for d in range(DC)]
            idx = wp.tile([P, TC], dt.int32, tag="idx", name="idx")
            wq = wp.tile([P, DC, D], dt.float32r, tag="wq", name="wq")
            wk = wp.tile([P, DC, D], dt.float32r, tag="wk", name="wk")
            wv = wp.tile([P, DC, D], dt.float32r, tag="wv", name="wv")
            lw = wp.tile([P, DC, D], dt.float32r, tag="lw", name="lw")
            lwT = wp.tile([P, DC, D], dt.float32r, tag="lwT", name="lwT")
            zjunk = pp.tile([P, 512], dt.float16, tag="zjunk", name="zjunk")

            # rings span iterations (same addresses every iteration)
            etf_ring = Ring(8)
            e16t_ring = Ring(5)
            s_ring = Ring(1)
            tp_ring = Ring(2)     # PSUM tag "tp": setup etp + attention tp/lt
            zp_ring = Ring(2)     # PSUM tag "zp": mps/gps/zp/op
            pex_ring = Ring(2)
            pt_ring = Ring(2)
            lp_ring = Ring(2)
            zr_ring = Ring(2)
            osb_ring = Ring(2)
            carry = {}

            for it in range(iters):
                _body(nc, mybir, dt,
                      dict(AF=AF, AX=AX, ALU=ALU, bass=bass, dep=dep,
                           identr=identr, ident16=ident16,
                           E16=E16, G16=G16, V16=V16, M16=M16, Wt16=Wt16,
                           Lbc=Lbc, Linv=Linv, Linv16=Linv16, linb=linb,
                           clfw=clfw, clfb=clfb, zsum=zsum, idx=idx,
                           wq=wq, wk=wk, wv=wv, lw=lw, lwT=lwT, zjunk=zjunk,
                           efp=efp, e16p=e16p, ppb=ppb, ptp=ptp, scr=scr,
                           sps=sps, tps=tps, zps=zps,
                           etf_ring=etf_ring, e16t_ring=e16t_ring,
                           s_ring=s_ring, tp_ring=tp_ring, zp_ring=zp_ring,
                           pex_ring=pex_ring, pt_ring=pt_ring,
                           lp_ring=lp_ring, zr_ring=zr_ring,
                           osb_ring=osb_ring, carry=carry),
                      x_d, emb_d, wq_d, wk_d, wv_d, lw_d, lb_d, cw_d, cb_d,
                      out_d.ap()[it:it + 1, :], fence_d.ap()[it])

    nc.compile()
    return nc


def _body(nc, mybir, dt, ctx,
          x_d, emb_d, wq_d, wk_d, wv_d, lw_d, lb_d, cw_d, cb_d, out_ap,
          fence_ap):
    AF, AX, ALU = ctx["AF"], ctx["AX"], ctx["ALU"]
    bass = ctx["bass"]
    dep = ctx["dep"]
    identr, ident16 = ctx["identr"], ctx["ident16"]
    E16, G16, V16, M16, Wt16 = (ctx[k] for k in ("E16", "G16", "V16", "M16", "Wt16"))
    Lbc, Linv, Linv16 = ctx["Lbc"], ctx["Linv"], ctx["Linv16"]
    linb, clfw, clfb, zsum = ctx["linb"], ctx["clfw"], ctx["clfb"], ctx["zsum"]
    idx, wq, wk, wv, lw, lwT = (ctx[k] for k in ("idx", "wq", "wk", "wv", "lw", "lwT"))
    zjunk = ctx["zjunk"]
    efp, e16p, ppb, ptp, scr = (ctx[k] for k in ("efp", "e16p", "ppb", "ptp", "scr"))
    sps, tps, zps = ctx["sps"], ctx["tps"], ctx["zps"]
    etf_ring, e16t_ring = ctx["etf_ring"], ctx["e16t_ring"]
    s_ring, tp_ring, zp_ring = ctx["s_ring"], ctx["tp_ring"], ctx["zp_ring"]
    pex_ring, pt_ring = ctx["pex_ring"], ctx["pt_ring"]
    lp_ring, zr_ring, osb_ring = ctx["lp_ring"], ctx["zr_ring"], ctx["osb_ring"]
    carry = ctx["carry"]

    # alternate DVE / ACT for PSUM->SBUF copies to balance engine load
    _cp = [0]

    def copy_ps(out, in_):
        if _cp[0] % 2 == 0:
            r = nc.vector.tensor_copy(out, in_)
        else:
            r = nc.scalar.copy(out, in_)
        _cp[0] += 1
        return r

    def guard_dma(dma, key):
        # this iteration's DMA must not overwrite a tile the previous
        # iteration still reads (DMA queues are unordered vs engines)
        if key in carry:
            prev = carry[key]
            for pinst in (prev if isinstance(prev, list) else [prev]):
                dep(dma, pinst, "cross-iter DMA WAR")
        return dma

    guard_dma(nc.sync.dma_start(out=linb[:],
                                in_=lb_d.ap().rearrange("(c p) -> p c", p=P)),
              "prev_end")
    guard_dma(nc.sync.dma_start(out=clfw[:],
                                in_=cw_d.ap().rearrange("(c p) -> p c", p=P)),
              "prev_end")
    guard_dma(nc.sync.dma_start(out=clfb[:], in_=cb_d.ap().unsqueeze(1)),
              "prev_end")
    guard_dma(guard_dma(nc.sync.dma_start(
        out=idx[:], in_=x_d.ap().rearrange("(p c) -> p c", c=TC)),
        "last_gathers"), "fence_dma")
    # chunked so the M matmuls can start after the first 512KB lands
    for dc in range(DC):
        for w_t, w_d in ((wq, wq_d), (wk, wk_d)):
            guard_dma(nc.sync.dma_start(
                out=w_t[:, dc, :],
                in_=w_d.ap()[dc * P:(dc + 1) * P, :]), "last_wread")
    for w_t, w_d in ((wv, wv_d), (lw, lw_d)):
        guard_dma(nc.sync.dma_start(
            out=w_t[:],
            in_=w_d.ap().rearrange("(c p) m -> p c m", p=P)), "last_wread")

    # ---------------- setup: M, Wt, gather+transpose, G, Vt ------------
    all_gathers = []

    def gather(g):
        tiles = [efp.tile([P, D], dt.float32, tag="etf", name="etf")
                 for _ in range(4)]
        for s in range(4):
            gi = nc.gpsimd.indirect_dma_start(
                out=tiles[s][:],
                out_offset=None,
                in_=emb_d.ap(),
                in_offset=bass.IndirectOffsetOnAxis(
                    ap=idx[:, g * 4 + s:g * 4 + s + 1], axis=0),
            )
            etf_ring.writer(gi)
            all_gathers.append(gi)
        if g == 3:
            carry["last_gathers"] = list(all_gathers)
        return tiles

    pending = [gather(0)]

    # M = W_q^T @ W_k   ([f,g], fp16)
    wread = []
    for fc in range(DC):
        mp = zps.tile([P, D], dt.float32, tag="zp", name="mps")
        for dc in range(DC):
            w = nc.tensor.matmul(mp[:], wq[:, dc, fc * P:(fc + 1) * P],
                                 wk[:, dc, :], start=(dc == 0),
                                 stop=(dc == DC - 1))
            if dc == 0:
                zp_ring.writer(w)
                if fc == 0 and "prev_end" in carry:
                    dep(w, carry["prev_end"], "cross-iter PSUM")
            if fc == DC - 1 and dc == DC - 1:
                wread.append(w)
        zp_ring.reader(copy_ps(M16[:, fc, :], mp[:]))

    # lin_w^T ([d2,a], f32r) then Wt = (lin_w @ W_v)^T ([d1,a], fp16)
    def do_wt():
        for c in range(DC):
            tp = zps.tile([P, 4, P], dt.float32r, tag="zp", name="wtp")
            for s in range(4):
                w = nc.tensor.transpose(tp[:, s, :],
                                        lw[:, s, c * P:(c + 1) * P],
                                        identr[:])
                if s == 0:
                    zp_ring.writer(w)
            zp_ring.reader(copy_ps(lwT[:, c, :], tp[:]))
        for c in range(DC):
            wtp = zps.tile([P, D], dt.float32, tag="zp", name="wtps")
            for d2 in range(DC):
                w = nc.tensor.matmul(wtp[:], wv[:, d2, c * P:(c + 1) * P],
                                     lwT[:, d2, :], start=(d2 == 0),
                                     stop=(d2 == DC - 1))
                if d2 == 0:
                    zp_ring.writer(w)
                if c == DC - 1 and d2 == DC - 1:
                    wread.append(w)
                    carry["last_wread"] = list(wread)
            zp_ring.reader(copy_ps(Wt16[:, c, :], wtp[:]))

    # per gather group: cast fp16, transpose into E16, then G and Vt
    def e_transpose(g):
        etf = pending.pop()
        e16t = [e16p.tile([P, D], dt.float16, tag="e16t", name="e16t")
                for _ in range(4)]
        for s in range(4):
            etf_ring.reader(e16t_ring.writer(
                nc.vector.tensor_copy(e16t[s][:], etf[s][:])))
        if g + 1 < 4:
            pending.append(gather(g + 1))
        ecopies = []
        for fc in range(DC):
            tp = tps.tile([P, 4, P], dt.float16, tag="tp", name="etp")
            for s in range(4):
                w = nc.tensor.transpose(tp[:, s, :],
                                        e16t[s][:, fc * P:(fc + 1) * P],
                                        ident16[:])
                if s == 0:
                    tp_ring.writer(w)
            ecopies.append(copy_ps(E16[:, fc, g * 512:(g + 1) * 512], tp[:]))
            tp_ring.reader(ecopies[-1])
        for s in range(4):
            e16t_ring.reader(ecopies)

    def gv_group(g):
        # G slice for this t-block
        for gc in range(DC):
            gps = zps.tile([P, 512], dt.float32, tag="zp", name="gps")
            for fc in range(DC):
                w = nc.tensor.matmul(gps[:], M16[:, fc, gc * P:(gc + 1) * P],
                                     E16[:, fc, g * 512:(g + 1) * 512],
                                     start=(fc == 0), stop=(fc == DC - 1))
                if fc == 0:
                    zp_ring.writer(w)
            zp_ring.reader(copy_ps(G16[:, gc, g * 512:(g + 1) * 512], gps[:]))
        # Vt chunks for this t-block
        for s in range(4):
            c = g * 4 + s
            vps = zps.tile([P, 512], dt.float32, tag="zp", name="vps")
            for d1 in range(DC):
                w = nc.tensor.matmul(vps[:], E16[:, d1, c * P:(c + 1) * P],
                                     Wt16[:, d1, :], start=(d1 == 0),
                                     stop=(d1 == DC - 1))
                if d1 == 0:
                    zp_ring.writer(w)
            zp_ring.reader(copy_ps(V16[:, c, :], vps[:]))

    e_transpose(0)
    do_wt()
    gv_group(0)
    for g in range(1, 4):
        e_transpose(g)
        gv_group(g)

    # setup-end fence: the NEXT body's input DMAs wait on this, limiting the
    # input stream to one body of lookahead (two-ahead corrupts; see module
    # docstring).  Placed here so next-body input DMAs overlap this body's
    # attention phase.
    fence = scr.tile([P, 8, 8], dt.float32, tag="fence", name="fence", bufs=1)
    fcp = [nc.vector.tensor_copy(fence[:, 0, :], E16[:, 0, 0:8]),
           nc.vector.tensor_copy(fence[:, 1, :], G16[:, 0, 0:8]),
           nc.vector.tensor_copy(fence[:, 2, :], V16[:, 0, 0:8]),
           nc.vector.tensor_copy(fence[:, 3, :], M16[:, 0, 0:8]),
           nc.vector.tensor_copy(fence[:, 4, :], Wt16[:, 0, 0:8])]
    carry["fence_dma"] = nc.sync.dma_start(
        out=fence_ap, in_=fence[0:1, :, :].rearrange("p a b -> p (a b)"))

    # ---------------- attention + folded linear + mean ----------------
    state = {}
    pv_last = {}

    def stage_scores(ic):
        S = sps.tile([P, T], dt.float32, tag="s", name="s")
        mx4 = scr.tile([P, NB], dt.float32, tag="mx4", name="mx4")
        for jb in range(NB):
            for gc in range(DC):
                w = nc.tensor.matmul(S[:, jb * 512:(jb + 1) * 512],
                                     G16[:, gc, ic * P:(ic + 1) * P],
                                     E16[:, gc, jb * 512:(jb + 1) * 512],
                                     start=(gc == 0), stop=(gc == DC - 1),
                                     skip_group_check=True)
                if jb == 0 and gc == 0:
                    s_ring.writer(w)
            nc.vector.tensor_reduce(mx4[:, jb:jb + 1],
                                    S[:, jb * 512:(jb + 1) * 512],
                                    axis=AX.X, op=ALU.max)
        state[ic] = (S, mx4)

    def stage_softmax(ic):
        S, mx4 = state.pop(ic)
        mx = scr.tile([P, 1], dt.float32, tag="mx", name="mx")
        nc.vector.tensor_reduce(mx[:], mx4[:], axis=AX.X, op=ALU.max)
        negb = scr.tile([P, 1], dt.float32, tag="negb", name="negb")
        nc.vector.tensor_scalar_mul(negb[:], mx[:], -float(SCALE))
        lp = scr.tile([P, 1], dt.float32, tag="lp", name="lp")
        Pex = ppb.tile([P, T], dt.float16, tag="pex", name="pex")
        ex = nc.scalar.activation(Pex[:], S[:], AF.Exp,
                                  bias=negb[:], scale=float(SCALE),
                                  accum_out=lp[:])
        s_ring.reader(ex)
        pex_ring.writer(ex)
        lp_ring.writer(ex)
        state[ic] = (Pex, lp)

    def stage_transpose(ic, PT):
        Pex, lp = state.pop(ic)
        lp_ring.reader(nc.vector.reciprocal(Linv[:, ic:ic + 1], lp[:]))
        nc.vector.tensor_copy(Linv16[:, ic:ic + 1], Linv[:, ic:ic + 1])
        s_i = ic % 4
        pcopies = []
        for g in range(TC // 4):
            tp = tps.tile([P, 4, P], dt.float16, tag="tp", name="tp")
            for s in range(4):
                jc = g * 4 + s
                w = nc.tensor.transpose(tp[:, s, :],
                                        Pex[:, jc * P:(jc + 1) * P],
                                        ident16[:])
                if s == 0:
                    tp_ring.writer(w)
            # DVE only: keeps the ACT queue clear for the next exp
            cp = nc.vector.tensor_copy(
                PT[:, g * 4:(g + 1) * 4, s_i * P:(s_i + 1) * P], tp[:])
            tp_ring.reader(cp)
            pcopies.append(cp)
            if s_i == 0 and g == 0:
                pt_ring.writer(cp)
        pex_ring.reader(pcopies)
        # broadcast 1/l along the free dim for the block-level scale
        lt = tps.tile([P, 4, P], dt.float16, tag="tp", name="lt")
        tp_ring.writer(nc.tensor.transpose(
            lt[:, 0, :], Linv16[:, ic:ic + 1].to_broadcast([P, P]),
            ident16[:]))
        tp_ring.reader(nc.scalar.copy(Lbc[:, ic * P:(ic + 1) * P],
                                      lt[:, 0, :]))

    def stage_pv_chunk(bo, dc, PT):
        zp = zps.tile([P, 512], dt.float32, tag="zp", name="zp")
        for jc in range(TC):
            w = nc.tensor.matmul(zp[:], V16[:, jc, dc * P:(dc + 1) * P],
                                 PT[:, jc, :],
                                 start=(jc == 0), stop=(jc == TC - 1))
            if jc == 0:
                zp_ring.writer(w)
        pv_last.setdefault(bo, []).append(w)
        if len(pv_last[bo]) == DC:
            pt_ring.reader(pv_last.pop(bo))
        zr = scr.tile([P, 512], dt.float16, tag="zr", name="zr")
        zrm = nc.vector.tensor_tensor(
            out=zr[:], in0=zp[:],
            in1=Lbc[:, bo * 512:(bo + 1) * 512], op=ALU.mult)
        zp_ring.reader(zrm)
        zr_ring.writer(zrm)
        zr_ring.reader(nc.scalar.activation(
            zjunk[:], zr[:], AF.Relu,
            bias=linb[:, dc:dc + 1], scale=1.0,
            accum_out=zsum[dc][:, bo:bo + 1]))

    # software pipeline: scores(ic+1) is emitted after PT(ic-1) and a PV
    # chunk of the previous block, so exp(ic) hides under PE work
    PTs = {}
    stage_scores(0)
    stage_softmax(0)
    for ic in range(TC):
        bo = ic // 4
        if ic % 4 == 0:
            PTs[bo] = ptp.tile([P, TC, 512], dt.float16, tag="pt", name="pt")
        if ic + 1 < TC:
            stage_scores(ic + 1)
            stage_softmax(ic + 1)
        stage_transpose(ic, PTs[bo])
        if bo > 0:
            stage_pv_chunk(bo - 1, ic % 4, PTs[bo - 1])
        if ic % 4 == 3 and bo > 0:
            PTs.pop(bo - 1)
    for dc in range(DC):
        stage_pv_chunk(NB - 1, dc, PTs[NB - 1])

    # ---------------- classifier ----------------
    ysum = [scr.tile([P, 1], dt.float32, tag=f"ys{d}", name=f"ys{d}")
            for d in range(DC)]
    for dc in range(DC):
        nc.vector.tensor_reduce(ysum[dc][:], zsum[dc][:], axis=AX.X,
                                op=ALU.add)
    op = zps.tile([P, 512], dt.float32, tag="zp", name="optp")
    for dc in range(DC):
        w = nc.tensor.matmul(op[:1, :1], clfw[:, dc:dc + 1], ysum[dc][:],
                             start=(dc == 0), stop=(dc == DC - 1))
        if dc == 0:
            zp_ring.writer(w)
    osb = scr.tile([1, 1], dt.float32, tag="osb", name="osb")
    sig = nc.scalar.activation(osb[:], op[:1, :1], AF.Sigmoid,
                               bias=clfb[:], scale=float(1.0 / T))
    zp_ring.reader(sig)
    osb_ring.writer(sig)
    carry["prev_end"] = sig
    osb_ring.reader(nc.sync.dma_start(out=out_ap, in_=osb[:]))


def _get_nc(iters=1, mm_dtype=None):
    key = (iters,)
    if key not in _COMPILED:
        _COMPILED[key] = _build(iters=iters)
    return _COMPILED[key]


def _in_maps(x, embed, W_q, W_k, W_v, lin_w, lin_b, clf_w, clf_b):
    x = np.ascontiguousarray(np.asarray(x).astype(np.int32))
    common = {
        "embed": np.ascontiguousarray(np.asarray(embed, np.float32)),
        "W_q": np.ascontiguousarray(np.asarray(W_q, np.float32)),
        "W_k": np.ascontiguousarray(np.asarray(W_k, np.float32)),
        "W_v": np.ascontiguousarray(np.asarray(W_v, np.float32)),
        "lin_w": np.ascontiguousarray(np.asarray(lin_w, np.float32)),
        "lin_b": np.ascontiguousarray(np.asarray(lin_b, np.float32).reshape(D)),
        "clf_w": np.ascontiguousarray(np.asarray(clf_w, np.float32).reshape(D)),
        "clf_b": np.ascontiguousarray(np.asarray(clf_b, np.float32).reshape(1)),
    }
    return [dict(common, x=x[c]) for c in range(N_CORES)]


def kernel(x, embed, W_q, W_k, W_v, lin_w, lin_b, clf_w, clf_b):
    from concourse.bass_utils import run_bass_kernel_spmd

    nc = _get_nc()
    in_maps = _in_maps(x, embed, W_q, W_k, W_v, lin_w, lin_b, clf_w, clf_b)
    res = run_bass_kernel_spmd(nc, in_maps, core_ids=list(range(N_CORES)))
    out = np.stack([res.results[c]["out"][0, 0] for c in range(N_CORES)])
    return out.reshape(B, 1).astype(np.float32)
